# revision 14
# baseline (speedup 1.0000x reference)
"""DistMult edge-scoring kernel for Trainium2 (8 NeuronCores, SPMD).

score[j] = sum_d emb_A[a_idx[j], d] * k[d] * emb_B[b_idx[j], d]
for 9E pairs: E positive edges, 4E head-corrupted, 4E tail-corrupted.

Strategy (v7, transposed dense bf16 stream, dual-queue DMA, batched DVE):
- HOST pre-gathers every pair's rows into one dense bf16 stream in a
  TRANSPOSED layout (d across the 128 partitions, pairs along the free
  dim): per edge only Ad=emb_A[ep0], Bd=emb_B[ep1]*k and the 8 corrupt
  rows stream in (10 rows x 256B = 2560B/edge, 32.1MB/core).  k is
  folded host-side and appears exactly once per score.
- Loads are batched (up to 7 groups = 2.29MB per DMA, small batches at
  the ends for fast ramp/drain) and ALTERNATE between the two HWDGE
  queues (nc.sync / nc.scalar) — a single queue caps at ~180GB/s on a
  loaded rig, two queues roughly double effective load bandwidth.
- DVE products for a whole batch issue as 2 tensor_tensor ops with a
  group-level (j) AP dimension: slots [pos|T0..T3] pair in0=[Bd|T...]
  with broadcast in1=Ad, slots [H0..H3] pair with in1=Bd.  Everything
  stays in the DVE 2x bf16 mode (~0.52ns/elem, ~59us/core total).
- PE reduces each 128x128 product slot over partitions with
  matmul(lhsT=slot, rhs=ones[128,1]) -> psum[128,9] per group; the Act
  engine evacuates psum into the SBUF score tile; scores stream out
  per-batch on the sync queue, overlapped with compute.
- Engine budget per core: DMA ~61-91us (rig-load dependent, bound),
  DVE ~61us, Act ~25us, PE < 60us.  Measured marginal steady-state
  61-96us/rep vs 189us for the v6 single-queue baseline.
"""

import numpy as np

# problem constants
N_A = 100000
N_B = 100000
D = 128
E = 100000
NEG = 4
NCORES = 8

P = 128
EC = E // NCORES          # edges per core (12500)
G = -(-EC // P)           # groups of 128 edges per core (98)
PAD = G * P               # padded edges per core (12544)
# DMA batch schedule: small batches at the ends for fast ramp/drain, big
# 7-group (2.29 MB) batches in steady state for descriptor amortization.
BATCHES = [1, 1, 2, 3] + [7] * 12 + [3, 2, 1, 1]
MAXB = max(BATCHES)
BUFS = 4
R = 9                     # scores per edge
W = 10 * P                # stream cols per group: [Ad|Bd|T0..T3|H0..H3]

_CACHED = {}


def _build_program(repeat=1):
    import concourse.tile as tile
    from concourse import bacc, mybir

    f32 = mybir.dt.float32
    bf16 = mybir.dt.bfloat16
    mult = mybir.AluOpType.mult

    nc = bacc.Bacc("TRN2", target_bir_lowering=False, debug=False,
                   num_devices=NCORES)
    x_d = nc.dram_tensor("x", [P, G * W], bf16, kind="ExternalInput").ap()
    # scores: [e-partition, g*9+j]; j 0-3: T_j, 4-7: H_{j-4}, 8: pos
    s_d = nc.dram_tensor("scores", [P, G * R], f32, kind="ExternalOutput").ap()

    with tile.TileContext(nc) as tc:
        with (
            tc.tile_pool(name="io", bufs=BUFS) as io_pool,
            tc.tile_pool(name="pr", bufs=3) as pr_pool,
            tc.tile_pool(name="ps", bufs=4, space="PSUM") as ps_pool,
            tc.tile_pool(name="on", bufs=1) as on_pool,
            tc.tile_pool(name="sc", bufs=1) as sc_pool,
        ):
            ones = on_pool.tile([P, 1], bf16)
            nc.vector.memset(ones[:], 1.0)
            sc = sc_pool.tile([P, G * R], f32)

            for _rep in range(repeat):
                _run_body(nc, x_d, s_d, io_pool, pr_pool, ps_pool, ones, sc)

    nc.compile()
    return nc


def _run_body(nc, x_d, s_d, io_pool, pr_pool, ps_pool, ones, sc):
    from concourse import mybir
    bf16 = mybir.dt.bfloat16
    f32 = mybir.dt.float32
    mult = mybir.AluOpType.mult
    b0 = 0
    for bi, n in enumerate(BATCHES):
        # load queue: ~60/40 split between the two HWDGE issuers (SP / ACT);
        # ACT also carries the psum-evac copies, so it gets the lighter share
        ldq = nc.sync if bi % 5 != 4 else nc.scalar
        x = io_pool.tile([P, MAXB * W], bf16, tag="x")
        ldq.dma_start(x[:, :n * W], x_d[:, b0 * W:(b0 + n) * W])

        prod = pr_pool.tile([P, MAXB * R * P], bf16, tag="pr")
        # batched products over all n groups at once (j level in the AP)
        x3 = x[:, :n * W].rearrange("p (j w) -> p j w", j=n)
        prod3 = prod[:, :n * R * P].rearrange(
            "p (j i e) -> p j i e", j=n, i=R)
        # slot r=0: pos = Bd*Ad; r=1-4: T_i*Ad  (in0 = [Bd|T0..T3])
        in0a = x3[:, :, P:6 * P]
        in1a = x3[:, :, 0:P].rearrange(
            "p j (one e) -> p j one e", one=1).broadcast_to([P, n, 5, P])
        nc.vector.tensor_tensor(out=prod3[:, :, 0:5, :], in0=in0a,
                                in1=in1a, op=mult)
        # slots r=5-8: H_i*Bd
        in0b = x3[:, :, 6 * P:10 * P]
        in1b = x3[:, :, P:2 * P].rearrange(
            "p j (one e) -> p j one e", one=1).broadcast_to([P, n, 4, P])
        nc.vector.tensor_tensor(out=prod3[:, :, 5:9, :], in0=in0b,
                                in1=in1b, op=mult)

        for j in range(n):
            g = b0 + j
            ps = ps_pool.tile([P, R], f32, tag="ps")
            for r in range(R):
                nc.tensor.matmul(ps[:, r:r + 1],
                                 prod[:, (j * R + r) * P:(j * R + r + 1) * P],
                                 ones[:], start=True, stop=True)
            nc.scalar.copy(out=sc[:, g * R:(g + 1) * R], in_=ps[:])
        # stream this batch's scores out while later batches compute
        nc.sync.dma_start(s_d[:, b0 * R:(b0 + n) * R],
                          sc[:, b0 * R:(b0 + n) * R])
        b0 += n
    assert b0 == G


def _host_prep(emb_A, emb_B, rel_kernel, edge_pos, head_batch, tail_batch):
    """Pre-gather pair rows into per-core transposed dense bf16 streams."""
    import ml_dtypes
    bf16 = ml_dtypes.bfloat16

    kv = np.asarray(rel_kernel, dtype=np.float32)[0]
    A16 = np.asarray(emb_A, dtype=np.float32).astype(bf16)
    Bk16 = (np.asarray(emb_B, dtype=np.float32) * kv[None, :]).astype(bf16)
    ep = np.asarray(edge_pos, dtype=np.int64)
    hb = np.asarray(head_batch, dtype=np.int64)
    tb = np.asarray(tail_batch, dtype=np.int64)

    in_maps = []
    outpos_cores = []
    for c in range(NCORES):
        sl = slice(c * EC, (c + 1) * EC)
        e0 = np.zeros(PAD, np.int64)
        e1 = np.zeros(PAD, np.int64)
        hbp = np.zeros((PAD, NEG), np.int64)
        tbp = np.zeros((PAD, NEG), np.int64)
        e0[:EC], e1[:EC] = ep[0, sl], ep[1, sl]
        hbp[:EC], tbp[:EC] = hb[sl], tb[sl]

        # ab[d, g, c, e]: c=0 Ad, c=1 Bd
        abr = np.stack([A16[e0], Bk16[e1]], axis=1)      # [PAD, 2, D]
        ab3 = abr.reshape(G, P, 2, D).transpose(3, 0, 2, 1).reshape(P, G, 2 * P)
        # ht[d, g, c, i, e]: c=0 T_i (Bk16[tb]), c=1 H_i (A16[hb])
        tt4 = Bk16[tbp.reshape(-1)].reshape(G, P, NEG, D)
        hh4 = A16[hbp.reshape(-1)].reshape(G, P, NEG, D)
        htr = np.stack([tt4, hh4], axis=2)               # [G, e, c, i, d]
        ht3 = htr.transpose(4, 0, 2, 3, 1).reshape(P, G, 8 * P)
        x = np.ascontiguousarray(
            np.concatenate([ab3, ht3], axis=2).reshape(P, G * W))
        in_maps.append({"x": x})

        # flat scores idx = (g*R + r)*128 + p ; p = edge-in-group
        # slot r=0: pos, r=1-4: T_(r-1), r=5-8: H_(r-5)
        gg, rr, pp = np.meshgrid(np.arange(G), np.arange(R), np.arange(P),
                                 indexing="ij")
        el = gg * P + pp
        eg = c * EC + el
        valid = el < EC
        ov = np.where(
            rr == 0, eg,
            np.where(rr < 5, 5 * E + eg * NEG + (rr - 1),
                     E + eg * NEG + (rr - 5)))
        outpos_cores.append(np.where(valid, ov, -1).reshape(-1))
    return in_maps, outpos_cores


def kernel(emb_A, emb_B, rel_kernel, edge_pos, head_batch, tail_batch):
    from concourse.bass_utils import run_bass_kernel_spmd

    in_maps, outpos_cores = _host_prep(
        emb_A, emb_B, rel_kernel, edge_pos, head_batch, tail_batch)

    if "nc" not in _CACHED:
        _CACHED["nc"] = _build_program()
    nc = _CACHED["nc"]
    _CACHED["in_maps"] = in_maps
    _CACHED["plan"] = "v6"

    res = run_bass_kernel_spmd(nc, in_maps, core_ids=list(range(NCORES)))
    _CACHED["last_results"] = res

    out = np.empty(9 * E, dtype=np.float32)
    for c in range(NCORES):
        ov = outpos_cores[c]
        fv = res.results[c]["scores"].T.reshape(-1)
        m = ov >= 0
        out[ov[m]] = fv[m]
    return out



# revision 15
# speedup vs baseline: 1.0462x; 1.0462x over previous
"""DistMult edge-scoring kernel for Trainium2 (8 NeuronCores, SPMD).

score[j] = sum_d emb_A[a_idx[j], d] * k[d] * emb_B[b_idx[j], d]
for 9E pairs: E positive edges, 4E head-corrupted, 4E tail-corrupted.

Strategy (v7, transposed dense bf16 stream, dual-queue DMA, batched DVE):
- HOST pre-gathers every pair's rows into one dense bf16 stream in a
  TRANSPOSED layout (d across the 128 partitions, pairs along the free
  dim): per edge only Ad=emb_A[ep0], Bd=emb_B[ep1]*k and the 8 corrupt
  rows stream in (10 rows x 256B = 2560B/edge, 32.1MB/core).  k is
  folded host-side and appears exactly once per score.
- Loads are batched (up to 7 groups = 2.29MB per DMA, small batches at
  the ends for fast ramp/drain) and ALTERNATE between the two HWDGE
  queues (nc.sync / nc.scalar) — a single queue caps at ~180GB/s on a
  loaded rig, two queues roughly double effective load bandwidth.
- DVE products for a whole batch issue as 2 tensor_tensor ops with a
  group-level (j) AP dimension: slots [pos|T0..T3] pair in0=[Bd|T...]
  with broadcast in1=Ad, slots [H0..H3] pair with in1=Bd.  Everything
  stays in the DVE 2x bf16 mode (~0.52ns/elem, ~59us/core total).
- PE reduces each 128x128 product slot over partitions with
  matmul(lhsT=slot, rhs=ones[128,1]) -> psum[128,9] per group; the Act
  engine evacuates psum into the SBUF score tile; scores stream out
  per-batch on the sync queue, overlapped with compute.
- Engine budget per core: DMA ~61-91us (rig-load dependent, bound),
  DVE ~61us, Act ~25us, PE < 60us.  Measured marginal steady-state
  61-96us/rep vs 189us for the v6 single-queue baseline.
"""

import numpy as np

# problem constants
N_A = 100000
N_B = 100000
D = 128
E = 100000
NEG = 4
NCORES = 8

P = 128
EC = E // NCORES          # edges per core (12500)
G = -(-EC // P)           # groups of 128 edges per core (98)
PAD = G * P               # padded edges per core (12544)
# DMA batch schedule: small batches at the ends for fast ramp/drain, big
# 7-group (2.29 MB) batches in steady state for descriptor amortization.
BATCHES = [1, 1, 2, 3] + [7] * 12 + [3, 2, 1, 1]
MAXB = max(BATCHES)
BUFS = 3
R = 9                     # scores per edge
W = 10 * P                # stream cols per group: [Ad|Bd|T0..T3|H0..H3]

_CACHED = {}


def _build_program(repeat=1):
    import concourse.tile as tile
    from concourse import bacc, mybir

    f32 = mybir.dt.float32
    bf16 = mybir.dt.bfloat16
    mult = mybir.AluOpType.mult

    nc = bacc.Bacc("TRN2", target_bir_lowering=False, debug=False,
                   num_devices=NCORES)
    x_d = nc.dram_tensor("x", [P, G * W], bf16, kind="ExternalInput").ap()
    # scores: [e-partition, g*9+j]; j 0-3: T_j, 4-7: H_{j-4}, 8: pos
    s_d = nc.dram_tensor("scores", [P, G * R], f32, kind="ExternalOutput").ap()

    with tile.TileContext(nc) as tc:
        with (
            tc.tile_pool(name="io", bufs=BUFS) as io_pool,
            tc.tile_pool(name="pr", bufs=3) as pr_pool,
            tc.tile_pool(name="ps", bufs=4, space="PSUM") as ps_pool,
            tc.tile_pool(name="on", bufs=1) as on_pool,
            tc.tile_pool(name="sc", bufs=1) as sc_pool,
        ):
            ones = on_pool.tile([P, 1], bf16)
            nc.vector.memset(ones[:], 1.0)
            sc = sc_pool.tile([P, G * R], f32)

            for _rep in range(repeat):
                _run_body(nc, x_d, s_d, io_pool, pr_pool, ps_pool, ones, sc)

    nc.compile()
    return nc


def _run_body(nc, x_d, s_d, io_pool, pr_pool, ps_pool, ones, sc):
    from concourse import mybir
    bf16 = mybir.dt.bfloat16
    f32 = mybir.dt.float32
    mult = mybir.AluOpType.mult
    b0 = 0
    for bi, n in enumerate(BATCHES):
        # load queue: ~60/40 split between the two HWDGE issuers (SP / ACT);
        # ACT also carries the psum-evac copies, so it gets the lighter share
        ldq = nc.sync if bi % 5 != 4 else nc.scalar
        x = io_pool.tile([P, MAXB * W], bf16, tag="x")
        ldq.dma_start(x[:, :n * W], x_d[:, b0 * W:(b0 + n) * W])

        prod = pr_pool.tile([P, MAXB * R * P], bf16, tag="pr")
        # batched products over all n groups at once (j level in the AP)
        x3 = x[:, :n * W].rearrange("p (j w) -> p j w", j=n)
        prod3 = prod[:, :n * R * P].rearrange(
            "p (j i e) -> p j i e", j=n, i=R)
        # slot r=0: pos = Bd*Ad; r=1-4: T_i*Ad  (in0 = [Bd|T0..T3])
        in0a = x3[:, :, P:6 * P]
        in1a = x3[:, :, 0:P].rearrange(
            "p j (one e) -> p j one e", one=1).broadcast_to([P, n, 5, P])
        nc.vector.tensor_tensor(out=prod3[:, :, 0:5, :], in0=in0a,
                                in1=in1a, op=mult)
        # slots r=5-8: H_i*Bd
        in0b = x3[:, :, 6 * P:10 * P]
        in1b = x3[:, :, P:2 * P].rearrange(
            "p j (one e) -> p j one e", one=1).broadcast_to([P, n, 4, P])
        nc.vector.tensor_tensor(out=prod3[:, :, 5:9, :], in0=in0b,
                                in1=in1b, op=mult)

        for j in range(n):
            g = b0 + j
            ps = ps_pool.tile([P, R], f32, tag="ps")
            for r in range(R):
                nc.tensor.matmul(ps[:, r:r + 1],
                                 prod[:, (j * R + r) * P:(j * R + r + 1) * P],
                                 ones[:], start=True, stop=True)
            nc.scalar.copy(out=sc[:, g * R:(g + 1) * R], in_=ps[:])
        # stream this batch's scores out while later batches compute
        nc.sync.dma_start(s_d[:, b0 * R:(b0 + n) * R],
                          sc[:, b0 * R:(b0 + n) * R])
        b0 += n
    assert b0 == G


def _host_prep(emb_A, emb_B, rel_kernel, edge_pos, head_batch, tail_batch):
    """Pre-gather pair rows into per-core transposed dense bf16 streams."""
    import ml_dtypes
    bf16 = ml_dtypes.bfloat16

    kv = np.asarray(rel_kernel, dtype=np.float32)[0]
    A16 = np.asarray(emb_A, dtype=np.float32).astype(bf16)
    Bk16 = (np.asarray(emb_B, dtype=np.float32) * kv[None, :]).astype(bf16)
    ep = np.asarray(edge_pos, dtype=np.int64)
    hb = np.asarray(head_batch, dtype=np.int64)
    tb = np.asarray(tail_batch, dtype=np.int64)

    in_maps = []
    outpos_cores = []
    for c in range(NCORES):
        sl = slice(c * EC, (c + 1) * EC)
        e0 = np.zeros(PAD, np.int64)
        e1 = np.zeros(PAD, np.int64)
        hbp = np.zeros((PAD, NEG), np.int64)
        tbp = np.zeros((PAD, NEG), np.int64)
        e0[:EC], e1[:EC] = ep[0, sl], ep[1, sl]
        hbp[:EC], tbp[:EC] = hb[sl], tb[sl]

        # ab[d, g, c, e]: c=0 Ad, c=1 Bd
        abr = np.stack([A16[e0], Bk16[e1]], axis=1)      # [PAD, 2, D]
        ab3 = abr.reshape(G, P, 2, D).transpose(3, 0, 2, 1).reshape(P, G, 2 * P)
        # ht[d, g, c, i, e]: c=0 T_i (Bk16[tb]), c=1 H_i (A16[hb])
        tt4 = Bk16[tbp.reshape(-1)].reshape(G, P, NEG, D)
        hh4 = A16[hbp.reshape(-1)].reshape(G, P, NEG, D)
        htr = np.stack([tt4, hh4], axis=2)               # [G, e, c, i, d]
        ht3 = htr.transpose(4, 0, 2, 3, 1).reshape(P, G, 8 * P)
        x = np.ascontiguousarray(
            np.concatenate([ab3, ht3], axis=2).reshape(P, G * W))
        in_maps.append({"x": x})

        # flat scores idx = (g*R + r)*128 + p ; p = edge-in-group
        # slot r=0: pos, r=1-4: T_(r-1), r=5-8: H_(r-5)
        gg, rr, pp = np.meshgrid(np.arange(G), np.arange(R), np.arange(P),
                                 indexing="ij")
        el = gg * P + pp
        eg = c * EC + el
        valid = el < EC
        ov = np.where(
            rr == 0, eg,
            np.where(rr < 5, 5 * E + eg * NEG + (rr - 1),
                     E + eg * NEG + (rr - 5)))
        outpos_cores.append(np.where(valid, ov, -1).reshape(-1))
    return in_maps, outpos_cores


def kernel(emb_A, emb_B, rel_kernel, edge_pos, head_batch, tail_batch):
    from concourse.bass_utils import run_bass_kernel_spmd

    in_maps, outpos_cores = _host_prep(
        emb_A, emb_B, rel_kernel, edge_pos, head_batch, tail_batch)

    if "nc" not in _CACHED:
        _CACHED["nc"] = _build_program()
    nc = _CACHED["nc"]
    _CACHED["in_maps"] = in_maps
    _CACHED["plan"] = "v6"

    res = run_bass_kernel_spmd(nc, in_maps, core_ids=list(range(NCORES)))
    _CACHED["last_results"] = res

    out = np.empty(9 * E, dtype=np.float32)
    for c in range(NCORES):
        ov = outpos_cores[c]
        fv = res.results[c]["scores"].T.reshape(-1)
        m = ov >= 0
        out[ov[m]] = fv[m]
    return out

